# revision 26
# baseline (speedup 1.0000x reference)
"""AudioXMMDiT self-attention Trainium2 kernel (8 NeuronCores, head-parallel).

Shapes (hardcoded): x [4, 4096, 1536] f32, cos/sin [4096, 64] f32,
w_qkv [1536, 4608] f32, b_qkv [4608] f32 -> out [4, 4096, 1536] f32.
24 heads of dim 64; 3 heads per core across 8 cores.

Per-core pipeline:
  phase 1: qkv = x @ w_shard (bf16 matmul, fp32 accum; bias is added by DVE
           during the PSUM->SBUF copy against a pre-broadcast bias tile),
           rms-norm (DVE-only rsqrt so the ACT exp table stays resident) +
           rope on q/k, transpose q/k to [64, seq] layout, v kept [seq, 64]
           with a ones column appended (yields the softmax denominator
           during p@v).
  phase 2: scoresT = kT.T-tile @ qT (PE) -> exp (ACT, scale=1/8, no max
           subtraction needed: |score|<=64 exactly since q,k are rms-normed)
           -> p@v_aug accumulate (PE) -> normalize by denominator row ->
           transpose back -> DMA out.  The PV matmuls lag the scores by TWO
           k-tiles so the in-order PE stream never breaks its back-to-back
           matmul pipeline on an ACT-exp semaphore (depth 1 still chased
           the exp; depth 2 measured 27us faster).

Engine occupancy at 2.15 ms/call: PE ~95% (2.04 ms, vs 1.76 ms fundamental
for the bf16 row counts; LDWEIGHTS is fully hidden, 15 us exposed), ACT
77% (1.68 ms exp), DVE 48%, PE idle 87 us (mostly phase-1 ph1-bank
serialization; moving ps_v to the tp2 bank traded it for attn-tail
contention and measured worse).  Tried and reverted: fp8 DoubleRow
(2x scores, 4x PV) — softmax-weight quantization noise propagates ~1:1 to
the output (the output is itself a weighted mean that shrinks by the same
sqrt(sum w^2) factor as the noise), costing +3-5e-2 relative error against
the 2e-2 gate; also a head-2 two-query-chunk regrouping (2.20 ms vs 2.15).
"""

import os
import sys
from contextlib import ExitStack

for _p in ("/opt/trn_rl_repo", "/opt/pypackages", "/root/.axon_site",
           "/root/.axon_site/_ro/trn_rl_repo", "/root/.axon_site/_ro/pypackages"):
    if os.path.isdir(_p) and _p not in sys.path:
        sys.path.append(_p)

import numpy as np
import ml_dtypes

import concourse.bass as bass
import concourse.tile as tile
import concourse.mybir as mybir
from concourse import bacc, masks
from concourse.bass_utils import run_bass_kernel_spmd

F32 = mybir.dt.float32
U32 = mybir.dt.uint32
BF16 = mybir.dt.bfloat16
AF = mybir.ActivationFunctionType
ALU = mybir.AluOpType

B, S, DIM = 4, 4096, 1536
NH, D = 24, 64
NCORES = 8
HPC = NH // NCORES          # heads per core = 3
CW = HPC * D                # per-core column width per q/k/v = 192
EPS = 1e-6
SCALE = D ** -0.5


def _derive(b, s):
    return dict(B=b, S=s, BS=b * s, KT=DIM // 128, ST_PER_B=s // 128,
                KTILES=s // 128, QC=min(512, s), NQC=s // min(512, s))


CFG = _derive(B, S)
BS = CFG["BS"]
KT = CFG["KT"]


def _emit(tc, cfg=None, phases=(1, 2)):
    no_pv = bool(int(os.environ.get("K_NO_PV", "0")))
    no_exp = bool(int(os.environ.get("K_NO_EXP", "0")))
    cfg = cfg or CFG
    B, S, BS = cfg["B"], cfg["S"], cfg["BS"]
    KT, ST_PER_B, KTILES = cfg["KT"], cfg["ST_PER_B"], cfg["KTILES"]
    QC, NQC = cfg["QC"], cfg["NQC"]
    nc = tc.nc
    xT_d = nc.declare_dram_parameter("xT", [DIM, BS], BF16, isOutput=False)
    w_d = nc.declare_dram_parameter("w", [DIM, 3 * CW], BF16, isOutput=False)
    b_d = nc.declare_dram_parameter("bias", [1, 3 * CW], BF16, isOutput=False)
    cos_d = nc.declare_dram_parameter("cos", [S, D], F32, isOutput=False)
    sin_d = nc.declare_dram_parameter("sin", [S, D], F32, isOutput=False)
    out_d = nc.declare_dram_parameter("out", [BS, CW], F32, isOutput=True)

    with ExitStack() as ctx:
        ep = ctx.enter_context

        # ---- constants -------------------------------------------------
        cpool = ep(tc.tile_pool(name="consts", bufs=1))
        w_sb = cpool.tile([128, KT, 3 * CW], BF16, tag="w")
        # per-kt DMAs (contiguous rows, no rearrange): the first qk matmul
        # only waits on slice 0 instead of the whole 1.8MB weight load
        for kt in range(KT):
            nc.sync.dma_start(w_sb[:, kt, :], w_d[kt * 128:(kt + 1) * 128, :])
        bias_sb = cpool.tile([1, 3 * CW], BF16, tag="bias")
        nc.sync.dma_start(bias_sb[:], b_d[:])
        ones_sb = cpool.tile([1, 128], BF16, tag="ones")
        nc.vector.memset(ones_sb[:], 1.0)
        identb = cpool.tile([128, 128], BF16, tag="identb")
        masks.make_identity(nc, identb[:])
        identf = cpool.tile([128, 128], F32, tag="identf")
        masks.make_identity(nc, identf[:])
        zero_c = cpool.tile([128, 1], F32, tag="zeroc")
        nc.vector.memset(zero_c[:], 0.0)
        nc.const_aps.aps[(F32, 0.0)] = zero_c[:]
        eps_c = cpool.tile([128, 1], F32, tag="epsc")
        nc.vector.memset(eps_c[:], EPS)
        nc.const_aps.aps[(F32, EPS)] = eps_c[:]
        magic_sb = cpool.tile([128, 1], U32, tag="magic")
        nc.vector.memset(magic_sb[:], 0x5F3759DF)
        # bias broadcast to all 128 partitions once (via PE) so the per-tile
        # bias add rides along the PSUM->SBUF copy on DVE instead of two
        # extra PE matmuls per tile.
        bias_bc = cpool.tile([128, 3 * CW], F32, tag="bias_bc")

        # ---- SBUF pools ------------------------------------------------
        xT_pool = ep(tc.tile_pool(name="xT", bufs=6))
        cs_pool = ep(tc.tile_pool(name="cs", bufs=4))
        qk_pool = ep(tc.tile_pool(name="qk", bufs=2))
        rope_pool = ep(tc.tile_pool(name="rope", bufs=2))
        stats_pool = ep(tc.tile_pool(name="stats", bufs=2))
        rqkb_pool = ep(tc.tile_pool(name="rqkb", bufs=2))
        qT_pool = ep(tc.tile_pool(name="qT", bufs=2))
        kT_pool = ep(tc.tile_pool(name="kT", bufs=2))
        v_pool = ep(tc.tile_pool(name="v", bufs=2))
        pT_pool = ep(tc.tile_pool(name="pT", bufs=4))
        osb_pool = ep(tc.tile_pool(name="osb", bufs=2))
        rc_pool = ep(tc.tile_pool(name="rc", bufs=2))
        out_pool = ep(tc.tile_pool(name="outp", bufs=3))

        # ---- PSUM pools (8 banks total) -------------------------------
        ps_s_pool = ep(tc.tile_pool(name="ps_s", bufs=2, space="PSUM"))  # 4 banks
        po_pool = ep(tc.tile_pool(name="po", bufs=1, space="PSUM"))      # 2 banks
        tp2_pool = ep(tc.tile_pool(name="tp2", bufs=1, space="PSUM"))    # 1 bank
        ph1_pool = ep(tc.tile_pool(name="ph1", bufs=1, space="PSUM"))    # 1 bank

        ps_b1 = ph1_pool.tile([128, 2 * CW], F32, tag="ph1")
        nc.tensor.matmul(ps_b1[:], ones_sb[:], bias_sb[:, 0:2 * CW],
                         start=True, stop=True)
        nc.vector.tensor_copy(bias_bc[:, 0:2 * CW], ps_b1[:])
        ps_b2 = ph1_pool.tile([128, CW], F32, tag="ph1")
        nc.tensor.matmul(ps_b2[:], ones_sb[:], bias_sb[:, 2 * CW:3 * CW],
                         start=True, stop=True)
        nc.vector.tensor_copy(bias_bc[:, 2 * CW:3 * CW], ps_b2[:])

        tiles = {}

        def alloc_batch(b):
            qTp = qT_pool.tile([128, S], BF16, tag="qTp")
            qTs = qT_pool.tile([64, S], BF16, tag="qTs")
            kTp = kT_pool.tile([128, S], BF16, tag="kTp")
            kTs = kT_pool.tile([64, S], BF16, tag="kTs")
            v = v_pool.tile([128, KTILES, HPC, D + 1], BF16, tag="v")
            tiles[b] = dict(qTp=qTp, qTs=qTs, kTp=kTp, kTs=kTs, v=v)
            nc.vector.memset(v[:, :, :, D:D + 1], 1.0)

        def phase1_tile(b, st):
            T = tiles[b]
            row0 = b * S + st * 128
            stsl = slice(st * 128, (st + 1) * 128)
            xT_sb = xT_pool.tile([128, KT, 128], BF16, tag="xT")
            nc.sync.dma_start(
                xT_sb[:],
                xT_d[:, row0:row0 + 128].rearrange("(kt p) s -> p kt s", p=128))
            cs_c = cs_pool.tile([128, D], F32, tag="cos")
            nc.sync.dma_start(cs_c[:], cos_d[stsl, :])
            cs_s = cs_pool.tile([128, D], F32, tag="sin")
            nc.sync.dma_start(cs_s[:], sin_d[stsl, :])

            # qk before v: the qk psum feeds the long chain (copy -> rms ->
            # rope -> transposes), so it must start first; v-first measured
            # 30us worse. With one ph1 bank the v matmuls serialize behind
            # the qk copy (~0.7us/tile) — both bank relocations and both
            # orderings measured worse, so this is the 8-bank optimum.
            ps_qk = ph1_pool.tile([128, 2 * CW], F32, tag="ph1")
            for kt in range(KT):
                nc.tensor.matmul(ps_qk[:], xT_sb[:, kt, :],
                                 w_sb[:, kt, 0:2 * CW],
                                 start=(kt == 0), stop=(kt == KT - 1))
            qk_sb = qk_pool.tile([128, 2 * CW], F32, tag="qk")
            nc.vector.tensor_tensor(qk_sb[:], ps_qk[:], bias_bc[:, 0:2 * CW],
                                    ALU.add)

            ps_v = ph1_pool.tile([128, CW], F32, tag="ph1")
            for kt in range(KT):
                nc.tensor.matmul(ps_v[:], xT_sb[:, kt, :],
                                 w_sb[:, kt, 2 * CW:3 * CW],
                                 start=(kt == 0), stop=(kt == KT - 1))
            nc.vector.tensor_tensor(
                T["v"][:, st, :, 0:D],
                ps_v[:].rearrange("p (h d) -> p h d", h=HPC),
                bias_bc[:, 2 * CW:3 * CW].rearrange("p (h d) -> p h d", h=HPC),
                ALU.add)

            ms = stats_pool.tile([128, 2 * HPC], F32, tag="ms")
            sq = stats_pool.tile([128, 2 * CW], F32, tag="sq")
            nc.vector.tensor_tensor(sq[:], qk_sb[:], qk_sb[:], ALU.mult)
            nc.vector.tensor_reduce(
                ms[:], sq[:].rearrange("p (i d) -> p i d", d=D),
                axis=mybir.AxisListType.X, op=ALU.add)
            # rsqrt on DVE only (bit-trick + 2 Newton steps) so phase 1 never
            # touches ACT tables (the softmax exp set stays resident).
            NH2 = 2 * HPC
            a = stats_pool.tile([128, NH2], F32, tag="rs_a")
            nc.vector.tensor_scalar(a[:], ms[:], 1.0 / D, EPS, ALU.mult, ALU.add)
            hneg = stats_pool.tile([128, NH2], F32, tag="rs_h")
            nc.vector.tensor_scalar_mul(hneg[:], a[:], -0.5)
            rr = stats_pool.tile([128, NH2], F32, tag="rr")
            nc.vector.tensor_scalar(
                rr[:].bitcast(U32), a[:].bitcast(U32), 1, None,
                ALU.logical_shift_right)
            nc.vector.tensor_tensor(
                rr[:].bitcast(U32),
                magic_sb[:, 0:1].broadcast_to([128, NH2]).bitcast(U32),
                rr[:].bitcast(U32), ALU.subtract)
            t1 = stats_pool.tile([128, NH2], F32, tag="rs_t")
            for _ in range(2):
                nc.vector.tensor_tensor(t1[:], rr[:], rr[:], ALU.mult)
                nc.vector.tensor_tensor(t1[:], t1[:], hneg[:], ALU.mult)
                nc.vector.tensor_scalar_add(t1[:], t1[:], 1.5)
                nc.vector.tensor_tensor(rr[:], rr[:], t1[:], ALU.mult)

            # rope: q-half on DVE, k-half on the otherwise-idle GpSimd —
            # the two chains are independent, halving the per-tile
            # elementwise latency that gates the phase-1 ramp.
            rqk = rope_pool.tile([128, 2 * CW], F32, tag="rope")
            c1 = cs_c[:, None, 0::2].broadcast_to([128, HPC, 32])
            s1 = cs_s[:, None, 0::2].broadcast_to([128, HPC, 32])
            tmp = rope_pool.tile([128, 2, HPC, 32], F32, tag="ropetmp")
            srca = qk_sb[:].rearrange("p (t h d) -> p t h d", t=2, h=HPC)
            dsta = rqk[:].rearrange("p (t h d) -> p t h d", t=2, h=HPC)
            rra = rr[:].rearrange("p (t h) -> p t h", t=2)
            for t, eng in ((0, nc.vector), (1, nc.gpsimd)):
                srcv, dstv, tm = srca[:, t], dsta[:, t], tmp[:, t]
                x1, x2 = srcv[..., 0::2], srcv[..., 1::2]
                o1, o2 = dstv[..., 0::2], dstv[..., 1::2]
                eng.tensor_tensor(o1, x1, c1, ALU.mult)
                eng.tensor_tensor(tm, x2, s1, ALU.mult)
                eng.tensor_tensor(o1, o1, tm, ALU.subtract)
                eng.tensor_tensor(o2, x2, c1, ALU.mult)
                eng.tensor_tensor(tm, x1, s1, ALU.mult)
                eng.tensor_tensor(o2, o2, tm, ALU.add)
                rrh = rra[:, t][..., None].broadcast_to([128, HPC, D])
                eng.tensor_tensor(dstv, dstv, rrh, ALU.mult)

            rqkb = rqkb_pool.tile([128, 2 * CW], BF16, tag="rqkb")
            nc.vector.tensor_copy(rqkb[:], rqk[:])

            for half, kp, ks in ((0, "qTp", "qTs"), (1, "kTp", "kTs")):
                o = half * CW
                ptp = tp2_pool.tile([128, 128], BF16, tag="tp2")
                nc.tensor.transpose(ptp[0:64, :], rqkb[:, o:o + D],
                                    identb[:], tile_position=(0, 0))
                nc.tensor.transpose(ptp[64:128, :], rqkb[:, o + D:o + 2 * D],
                                    identb[:], tile_position=(0, 64))
                nc.vector.tensor_copy(T[kp][:, stsl], ptp[:])
                pts = tp2_pool.tile([64, 128], BF16, tag="tp2")
                nc.tensor.transpose(pts[:], rqkb[:, o + 2 * D:o + 3 * D],
                                    identb[:])
                nc.vector.tensor_copy(T[ks][:, stsl], pts[:])

        def attn_tail(b, ps_o, slots):
            """slots: list of (head, qc) for each slot of ps_o [65, n, QC]."""
            nh = len(slots)
            o_sb = osb_pool.tile([D + 1, nh, QC], F32, tag="osb")
            nc.vector.tensor_copy(o_sb[:], ps_o[:])
            for i, (h, qc) in enumerate(slots):
                for j in range(QC // 128):
                    tpo = tp2_pool.tile([128, D + 1], F32, tag="tp2")
                    nc.tensor.transpose(
                        tpo[:], o_sb[:, i, j * 128:(j + 1) * 128],
                        identf[0:D + 1, 0:D + 1])
                    rc = rc_pool.tile([128, 1], F32, tag="rc")
                    nc.vector.reciprocal(rc[:], tpo[:, D:D + 1])
                    out_sb = out_pool.tile([128, D], F32, tag="out")
                    nc.vector.tensor_scalar_mul(out_sb[:], tpo[:, 0:D], rc[:])
                    r0 = b * S + qc * QC + j * 128
                    nc.sync.dma_start(
                        out_d[r0:r0 + 128, h * D:(h + 1) * D], out_sb[:])

        def phase2_pair_chunk(b, qc):
            """Software-pipelined: exp/PV run one k-tile behind the scores
            matmuls so the in-order PE never waits on the ACT exp result."""
            T = tiles[b]
            qcs = slice(qc * QC, (qc + 1) * QC)
            ps_o = None if (no_pv or no_exp) else po_pool.tile(
                [D + 1, 2, QC], F32, tag="po")

            def exp_of(ps_s):
                pT = pT_pool.tile([128, 2, QC], BF16, tag="pT")
                nc.scalar.activation(
                    pT[:].rearrange("p a q -> p (a q)"),
                    ps_s[:].rearrange("p a q -> p (a q)"),
                    AF.Exp, scale=SCALE)
                return pT

            def pv(pT, kt, i):
                nc.tensor.matmul(
                    ps_o[:, i, :], T["v"][:, kt, i, :], pT[:, i, :],
                    start=(kt == 0), stop=(kt == KTILES - 1))

            # PV lags the scores by TWO k-tiles: by the time the in-order PE
            # reaches pv(kt-2), its exp finished long ago, so the PE never
            # breaks its back-to-back matmul stream on an ACT semaphore.
            pend = []
            for kt in range(KTILES):
                kts = slice(kt * 128, (kt + 1) * 128)
                ps_s = ps_s_pool.tile([128, 2, QC], F32, tag="ps_s")
                nc.tensor.matmul(ps_s[:, 0, :], T["kTp"][0:64, kts],
                                 T["qTp"][0:64, qcs], start=True, stop=True)
                nc.tensor.matmul(ps_s[:, 1, :], T["kTp"][64:128, kts],
                                 T["qTp"][64:128, qcs], start=True, stop=True)
                if no_exp:
                    continue
                pend.append((exp_of(ps_s), kt))
                if len(pend) > 2 and not no_pv:
                    pT_, kt_ = pend.pop(0)
                    pv(pT_, kt_, 0)
                    pv(pT_, kt_, 1)
            if not no_pv:
                for pT_, kt_ in pend:
                    pv(pT_, kt_, 0)
                    pv(pT_, kt_, 1)
            if not (no_pv or no_exp):
                attn_tail(b, ps_o, [(0, qc), (1, qc)])

        def phase2_single_chunk(b, qc):
            T = tiles[b]
            qcs = slice(qc * QC, (qc + 1) * QC)
            ps_o = None if (no_pv or no_exp) else po_pool.tile(
                [D + 1, 1, QC], F32, tag="po")

            def exp_of(ps_s):
                pT = pT_pool.tile([128, 2, QC], BF16, tag="pT")
                nc.scalar.activation(
                    pT[:].rearrange("p a q -> p (a q)"),
                    ps_s[:].rearrange("p a q -> p (a q)"),
                    AF.Exp, scale=SCALE)
                return pT

            def pv(pT, g, i):
                kt = 2 * g + i
                nc.tensor.matmul(
                    ps_o[:, 0, :], T["v"][:, kt, 2, :], pT[:, i, :],
                    start=(kt == 0), stop=(kt == KTILES - 1))

            pend = []
            for g in range(KTILES // 2):
                ps_s = ps_s_pool.tile([128, 2, QC], F32, tag="ps_s")
                for i in range(2):
                    kt = 2 * g + i
                    nc.tensor.matmul(
                        ps_s[:, i, :], T["kTs"][:, kt * 128:(kt + 1) * 128],
                        T["qTs"][:, qcs], start=True, stop=True)
                if no_exp:
                    continue
                pend.append((exp_of(ps_s), g))
                if len(pend) > 2 and not no_pv:
                    pT_, g_ = pend.pop(0)
                    pv(pT_, g_, 0)
                    pv(pT_, g_, 1)
            if not no_pv:
                for pT_, g_ in pend:
                    pv(pT_, g_, 0)
                    pv(pT_, g_, 1)
            if not (no_pv or no_exp):
                attn_tail(b, ps_o, [(2, qc)])

        # ---- interleaved emission: phase2(b) with phase1(b+1) ----------
        do1, do2 = (1 in phases), (2 in phases)
        alloc_batch(0)
        if do1:
            for st in range(ST_PER_B):
                phase1_tile(0, st)
        for b in range(B):
            chunks = []
            if do2:
                for qc in range(NQC):
                    chunks.append(lambda b=b, qc=qc: phase2_pair_chunk(b, qc))
                    chunks.append(lambda b=b, qc=qc: phase2_single_chunk(b, qc))
            nxt = []
            if b + 1 < B:
                alloc_batch(b + 1)
                if do1:
                    nxt = [lambda st=st: phase1_tile(b + 1, st)
                           for st in range(ST_PER_B)]
            if not chunks:
                for t in nxt:
                    t()
                continue
            per = (len(nxt) + len(chunks) - 1) // len(chunks) if nxt else 0
            for i, c in enumerate(chunks):
                c()
                for t in nxt[i * per:(i + 1) * per]:
                    t()


_CACHE = {}


def _get_program():
    if "nc" not in _CACHE:
        nc = bacc.Bacc("TRN2", target_bir_lowering=False, debug=False,
                       num_devices=NCORES)
        with tile.TileContext(nc) as tc:
            _emit(tc)
        nc.compile()
        _CACHE["nc"] = nc
    return _CACHE["nc"]


def _get_runner():
    """Build (once) a cached sharded-jit callable over 8 cores."""
    if "runner" in _CACHE:
        return _CACHE["runner"]
    nc = _get_program()
    import jax
    from jax.sharding import Mesh, PartitionSpec
    try:
        from jax.experimental.shard_map import shard_map
    except ImportError:
        from jax.shard_map import shard_map
    from concourse import bass2jax
    bass2jax.install_neuronx_cc_hook()

    part_name = (nc.partition_id_tensor.name
                 if nc.partition_id_tensor is not None else None)
    in_names, out_names, out_avals, zero_shapes = [], [], [], []
    for alloc in nc.m.functions[0].allocations:
        if not isinstance(alloc, mybir.MemoryLocationSet):
            continue
        name = alloc.memorylocations[0].name
        if alloc.kind == "ExternalInput":
            if name != part_name:
                in_names.append(name)
        elif alloc.kind == "ExternalOutput":
            out_names.append(name)
            shape = tuple(alloc.tensor_shape)
            dtype = mybir.dt.np(alloc.dtype)
            out_avals.append(jax.core.ShapedArray(shape, dtype))
            zero_shapes.append((shape, dtype))
    n_params = len(in_names)
    all_names = in_names + out_names
    if part_name is not None:
        all_names = all_names + [part_name]

    def _body(*args):
        operands = list(args)
        if part_name is not None:
            operands.append(bass2jax.partition_id_tensor())
        outs = bass2jax._bass_exec_p.bind(
            *operands,
            out_avals=tuple(out_avals),
            in_names=tuple(all_names),
            out_names=tuple(out_names),
            lowering_input_output_aliases=(),
            sim_require_finite=True,
            sim_require_nnan=True,
            nc=nc,
        )
        return tuple(outs)

    devices = jax.devices()[:NCORES]
    mesh = Mesh(np.asarray(devices), ("core",))
    n_outs = len(out_names)
    donate = tuple(range(n_params, n_params + n_outs))
    in_specs = (PartitionSpec("core"),) * (n_params + n_outs)
    out_specs = (PartitionSpec("core"),) * n_outs
    sharded = jax.jit(
        shard_map(_body, mesh=mesh, in_specs=in_specs, out_specs=out_specs,
                  check_rep=False),
        donate_argnums=donate, keep_unused=True)
    _CACHE["runner"] = (sharded, in_names, out_names, zero_shapes, mesh)
    return _CACHE["runner"]


def _prep_inputs(x, cos, sin, w_qkv, b_qkv):
    """Per-core shards, concatenated along axis 0 (shard_map layout)."""
    xT = np.ascontiguousarray(x.reshape(BS, DIM).T).astype(ml_dtypes.bfloat16)
    cos32 = np.ascontiguousarray(cos, dtype=np.float32)
    sin32 = np.ascontiguousarray(sin, dtype=np.float32)
    in_maps = []
    for c in range(NCORES):
        c0 = c * CW
        w_shard = np.concatenate(
            [w_qkv[:, c0:c0 + CW], w_qkv[:, DIM + c0:DIM + c0 + CW],
             w_qkv[:, 2 * DIM + c0:2 * DIM + c0 + CW]], axis=1)
        b_shard = np.concatenate(
            [b_qkv[c0:c0 + CW], b_qkv[DIM + c0:DIM + c0 + CW],
             b_qkv[2 * DIM + c0:2 * DIM + c0 + CW]])[None, :]
        in_maps.append({
            "xT": xT,
            "w": np.ascontiguousarray(w_shard).astype(ml_dtypes.bfloat16),
            "bias": np.ascontiguousarray(b_shard).astype(ml_dtypes.bfloat16),
            "cos": cos32,
            "sin": sin32,
        })
    return in_maps


def _run(in_maps):
    sharded, in_names, out_names, zero_shapes, mesh = _get_runner()
    concat_in = [
        np.concatenate([in_maps[c][name] for c in range(NCORES)], axis=0)
        for name in in_names]
    zeros = [np.zeros((NCORES * s[0], *s[1:]), dt) for s, dt in zero_shapes]
    out_arrs = sharded(*concat_in, *zeros)
    res = {}
    for i, name in enumerate(out_names):
        arr = np.asarray(out_arrs[i])
        res[name] = arr.reshape(NCORES, arr.shape[0] // NCORES, *arr.shape[1:])
    return res


def kernel(x, cos, sin, w_qkv, b_qkv):
    in_maps = _prep_inputs(x, cos, sin, w_qkv, b_qkv)
    res = _run(in_maps)
    out = np.concatenate([res["out"][c] for c in range(NCORES)], axis=1)
    return np.ascontiguousarray(out.reshape(B, S, DIM), dtype=np.float32)



# revision 27
# speedup vs baseline: 1.0824x; 1.0824x over previous
"""AudioXMMDiT self-attention Trainium2 kernel (8 NeuronCores, head-parallel).

Shapes (hardcoded): x [4, 4096, 1536] f32, cos/sin [4096, 64] f32,
w_qkv [1536, 4608] f32, b_qkv [4608] f32 -> out [4, 4096, 1536] f32.
24 heads of dim 64; 3 heads per core across 8 cores.

Per-core pipeline:
  phase 1: qkv = x @ w_shard (bf16 matmul, fp32 accum; bias is added by DVE
           during the PSUM->SBUF copy against a pre-broadcast bias tile),
           rms-norm (DVE-only rsqrt so the ACT exp table stays resident) +
           rope on q/k, transpose q/k to [64, seq] layout, v kept [seq, 64]
           with a ones column appended (yields the softmax denominator
           during p@v).
  phase 2: scoresT = kT.T-tile @ qT (PE) -> exp (ACT, scale=1/8, no max
           subtraction needed: |score|<=64 exactly since q,k are rms-normed)
           -> p@v_aug accumulate (PE) -> normalize by denominator row ->
           transpose back -> DMA out.  The PV matmuls lag the scores by TWO
           k-tiles so the in-order PE stream never breaks its back-to-back
           matmul pipeline on an ACT-exp semaphore (depth 1 still chased
           the exp; depth 2 measured 27us faster).

Engine occupancy at 2.15 ms/call: PE ~95% (2.04 ms, vs 1.76 ms fundamental
for the bf16 row counts; LDWEIGHTS is fully hidden, 15 us exposed), ACT
77% (1.68 ms exp), DVE 48%, PE idle 87 us (mostly phase-1 ph1-bank
serialization; moving ps_v to the tp2 bank traded it for attn-tail
contention and measured worse).  Tried and reverted: fp8 DoubleRow
(2x scores, 4x PV) — softmax-weight quantization noise propagates ~1:1 to
the output (the output is itself a weighted mean that shrinks by the same
sqrt(sum w^2) factor as the noise), costing +3-5e-2 relative error against
the 2e-2 gate; also a head-2 two-query-chunk regrouping (2.20 ms vs 2.15).
"""

import os
import sys
from contextlib import ExitStack

for _p in ("/opt/trn_rl_repo", "/opt/pypackages", "/root/.axon_site",
           "/root/.axon_site/_ro/trn_rl_repo", "/root/.axon_site/_ro/pypackages"):
    if os.path.isdir(_p) and _p not in sys.path:
        sys.path.append(_p)

import numpy as np
import ml_dtypes

import concourse.bass as bass
import concourse.tile as tile
import concourse.mybir as mybir
from concourse import bacc, masks
from concourse.bass_utils import run_bass_kernel_spmd

F32 = mybir.dt.float32
U32 = mybir.dt.uint32
BF16 = mybir.dt.bfloat16
AF = mybir.ActivationFunctionType
ALU = mybir.AluOpType

B, S, DIM = 4, 4096, 1536
NH, D = 24, 64
NCORES = 8
HPC = NH // NCORES          # heads per core = 3
CW = HPC * D                # per-core column width per q/k/v = 192
EPS = 1e-6
SCALE = D ** -0.5


def _derive(b, s):
    return dict(B=b, S=s, BS=b * s, KT=DIM // 128, ST_PER_B=s // 128,
                KTILES=s // 128, QC=min(512, s), NQC=s // min(512, s))


CFG = _derive(B, S)
BS = CFG["BS"]
KT = CFG["KT"]


def _emit(tc, cfg=None, phases=(1, 2)):
    no_pv = bool(int(os.environ.get("K_NO_PV", "0")))
    no_exp = bool(int(os.environ.get("K_NO_EXP", "0")))
    cfg = cfg or CFG
    B, S, BS = cfg["B"], cfg["S"], cfg["BS"]
    KT, ST_PER_B, KTILES = cfg["KT"], cfg["ST_PER_B"], cfg["KTILES"]
    QC, NQC = cfg["QC"], cfg["NQC"]
    nc = tc.nc
    xT_d = nc.declare_dram_parameter("xT", [DIM, BS], BF16, isOutput=False)
    w_d = nc.declare_dram_parameter("w", [DIM, 3 * CW], BF16, isOutput=False)
    b_d = nc.declare_dram_parameter("bias", [1, 3 * CW], BF16, isOutput=False)
    cos_d = nc.declare_dram_parameter("cos", [S, D], F32, isOutput=False)
    sin_d = nc.declare_dram_parameter("sin", [S, D], F32, isOutput=False)
    out_d = nc.declare_dram_parameter("out", [BS, CW], F32, isOutput=True)

    with ExitStack() as ctx:
        ep = ctx.enter_context

        # ---- constants -------------------------------------------------
        cpool = ep(tc.tile_pool(name="consts", bufs=1))
        w_sb = cpool.tile([128, KT, 3 * CW], BF16, tag="w")
        # per-kt DMAs (contiguous rows, no rearrange): the first qk matmul
        # only waits on slice 0 instead of the whole 1.8MB weight load
        for kt in range(KT):
            nc.sync.dma_start(w_sb[:, kt, :], w_d[kt * 128:(kt + 1) * 128, :])
        bias_sb = cpool.tile([1, 3 * CW], BF16, tag="bias")
        nc.sync.dma_start(bias_sb[:], b_d[:])
        ones_sb = cpool.tile([1, 128], BF16, tag="ones")
        nc.vector.memset(ones_sb[:], 1.0)
        identb = cpool.tile([128, 128], BF16, tag="identb")
        masks.make_identity(nc, identb[:])
        identf = cpool.tile([128, 128], F32, tag="identf")
        masks.make_identity(nc, identf[:])
        zero_c = cpool.tile([128, 1], F32, tag="zeroc")
        nc.vector.memset(zero_c[:], 0.0)
        nc.const_aps.aps[(F32, 0.0)] = zero_c[:]
        eps_c = cpool.tile([128, 1], F32, tag="epsc")
        nc.vector.memset(eps_c[:], EPS)
        nc.const_aps.aps[(F32, EPS)] = eps_c[:]
        magic_sb = cpool.tile([128, 1], U32, tag="magic")
        nc.vector.memset(magic_sb[:], 0x5F3759DF)
        # bias broadcast to all 128 partitions once (via PE) so the per-tile
        # bias add rides along the PSUM->SBUF copy on DVE instead of two
        # extra PE matmuls per tile.
        bias_bc = cpool.tile([128, 3 * CW], F32, tag="bias_bc")

        # ---- SBUF pools ------------------------------------------------
        xT_pool = ep(tc.tile_pool(name="xT", bufs=6))
        cs_pool = ep(tc.tile_pool(name="cs", bufs=4))
        qk_pool = ep(tc.tile_pool(name="qk", bufs=2))
        rope_pool = ep(tc.tile_pool(name="rope", bufs=2))
        stats_pool = ep(tc.tile_pool(name="stats", bufs=2))
        rqkb_pool = ep(tc.tile_pool(name="rqkb", bufs=2))
        qT_pool = ep(tc.tile_pool(name="qT", bufs=2))
        kT_pool = ep(tc.tile_pool(name="kT", bufs=2))
        v_pool = ep(tc.tile_pool(name="v", bufs=2))
        pT_pool = ep(tc.tile_pool(name="pT", bufs=4))
        osb_pool = ep(tc.tile_pool(name="osb", bufs=2))
        rc_pool = ep(tc.tile_pool(name="rc", bufs=2))
        out_pool = ep(tc.tile_pool(name="outp", bufs=3))

        # ---- PSUM pools (8 banks total) -------------------------------
        ps_s_pool = ep(tc.tile_pool(name="ps_s", bufs=2, space="PSUM"))  # 4 banks
        po_pool = ep(tc.tile_pool(name="po", bufs=1, space="PSUM"))      # 2 banks
        tp2_pool = ep(tc.tile_pool(name="tp2", bufs=1, space="PSUM"))    # 1 bank
        ph1_pool = ep(tc.tile_pool(name="ph1", bufs=1, space="PSUM"))    # 1 bank

        ps_b1 = ph1_pool.tile([128, 2 * CW], F32, tag="ph1")
        nc.tensor.matmul(ps_b1[:], ones_sb[:], bias_sb[:, 0:2 * CW],
                         start=True, stop=True)
        nc.vector.tensor_copy(bias_bc[:, 0:2 * CW], ps_b1[:])
        ps_b2 = ph1_pool.tile([128, CW], F32, tag="ph1")
        nc.tensor.matmul(ps_b2[:], ones_sb[:], bias_sb[:, 2 * CW:3 * CW],
                         start=True, stop=True)
        nc.vector.tensor_copy(bias_bc[:, 2 * CW:3 * CW], ps_b2[:])

        tiles = {}

        def alloc_batch(b):
            qTp = qT_pool.tile([128, S], BF16, tag="qTp")
            qTs = qT_pool.tile([64, S], BF16, tag="qTs")
            kTp = kT_pool.tile([128, S], BF16, tag="kTp")
            kTs = kT_pool.tile([64, S], BF16, tag="kTs")
            v = v_pool.tile([128, KTILES, HPC, D + 1], BF16, tag="v")
            tiles[b] = dict(qTp=qTp, qTs=qTs, kTp=kTp, kTs=kTs, v=v)
            nc.vector.memset(v[:, :, :, D:D + 1], 1.0)

        def phase1_tile(b, st):
            T = tiles[b]
            row0 = b * S + st * 128
            stsl = slice(st * 128, (st + 1) * 128)
            xT_sb = xT_pool.tile([128, KT, 128], BF16, tag="xT")
            nc.sync.dma_start(
                xT_sb[:],
                xT_d[:, row0:row0 + 128].rearrange("(kt p) s -> p kt s", p=128))
            cs_c = cs_pool.tile([128, D], F32, tag="cos")
            nc.sync.dma_start(cs_c[:], cos_d[stsl, :])
            cs_s = cs_pool.tile([128, D], F32, tag="sin")
            nc.sync.dma_start(cs_s[:], sin_d[stsl, :])

            # qk before v: the qk psum feeds the long chain (copy -> rms ->
            # rope -> transposes), so it must start first; v-first measured
            # 30us worse. With one ph1 bank the v matmuls serialize behind
            # the qk copy (~0.7us/tile) — both bank relocations and both
            # orderings measured worse, so this is the 8-bank optimum.
            ps_qk = ph1_pool.tile([128, 2 * CW], F32, tag="ph1")
            for kt in range(KT):
                nc.tensor.matmul(ps_qk[:], xT_sb[:, kt, :],
                                 w_sb[:, kt, 0:2 * CW],
                                 start=(kt == 0), stop=(kt == KT - 1))
            qk_sb = qk_pool.tile([128, 2 * CW], F32, tag="qk")
            nc.vector.tensor_tensor(qk_sb[:], ps_qk[:], bias_bc[:, 0:2 * CW],
                                    ALU.add)

            ps_v = ph1_pool.tile([128, CW], F32, tag="ph1")
            for kt in range(KT):
                nc.tensor.matmul(ps_v[:], xT_sb[:, kt, :],
                                 w_sb[:, kt, 2 * CW:3 * CW],
                                 start=(kt == 0), stop=(kt == KT - 1))
            nc.vector.tensor_tensor(
                T["v"][:, st, :, 0:D],
                ps_v[:].rearrange("p (h d) -> p h d", h=HPC),
                bias_bc[:, 2 * CW:3 * CW].rearrange("p (h d) -> p h d", h=HPC),
                ALU.add)

            ms = stats_pool.tile([128, 2 * HPC], F32, tag="ms")
            sq = stats_pool.tile([128, 2 * CW], F32, tag="sq")
            nc.vector.tensor_tensor(sq[:], qk_sb[:], qk_sb[:], ALU.mult)
            nc.vector.tensor_reduce(
                ms[:], sq[:].rearrange("p (i d) -> p i d", d=D),
                axis=mybir.AxisListType.X, op=ALU.add)
            # rsqrt on DVE only (bit-trick + 2 Newton steps) so phase 1 never
            # touches ACT tables (the softmax exp set stays resident).
            NH2 = 2 * HPC
            a = stats_pool.tile([128, NH2], F32, tag="rs_a")
            nc.vector.tensor_scalar(a[:], ms[:], 1.0 / D, EPS, ALU.mult, ALU.add)
            hneg = stats_pool.tile([128, NH2], F32, tag="rs_h")
            nc.vector.tensor_scalar_mul(hneg[:], a[:], -0.5)
            rr = stats_pool.tile([128, NH2], F32, tag="rr")
            nc.vector.tensor_scalar(
                rr[:].bitcast(U32), a[:].bitcast(U32), 1, None,
                ALU.logical_shift_right)
            nc.vector.tensor_tensor(
                rr[:].bitcast(U32),
                magic_sb[:, 0:1].broadcast_to([128, NH2]).bitcast(U32),
                rr[:].bitcast(U32), ALU.subtract)
            t1 = stats_pool.tile([128, NH2], F32, tag="rs_t")
            for _ in range(2):
                nc.vector.tensor_tensor(t1[:], rr[:], rr[:], ALU.mult)
                nc.vector.tensor_tensor(t1[:], t1[:], hneg[:], ALU.mult)
                nc.vector.tensor_scalar_add(t1[:], t1[:], 1.5)
                nc.vector.tensor_tensor(rr[:], rr[:], t1[:], ALU.mult)

            # rope on DVE only: splitting the k-half onto GpSimd measured
            # +180us (the Q7 software engine is slow on the stride-2 rope
            # access patterns and adds cross-engine sem hops before the
            # transposes).
            rqk = rope_pool.tile([128, 2 * CW], F32, tag="rope")
            c1 = cs_c[:, None, None, 0::2].broadcast_to([128, 2, HPC, 32])
            s1 = cs_s[:, None, None, 0::2].broadcast_to([128, 2, HPC, 32])
            srcv = qk_sb[:].rearrange("p (t h d) -> p t h d", t=2, h=HPC)
            dstv = rqk[:].rearrange("p (t h d) -> p t h d", t=2, h=HPC)
            x1, x2 = srcv[..., 0::2], srcv[..., 1::2]
            o1, o2 = dstv[..., 0::2], dstv[..., 1::2]
            tmp = rope_pool.tile([128, 2, HPC, 32], F32, tag="ropetmp")
            nc.vector.tensor_tensor(o1, x1, c1, ALU.mult)
            nc.vector.tensor_tensor(tmp[:], x2, s1, ALU.mult)
            nc.vector.tensor_tensor(o1, o1, tmp[:], ALU.subtract)
            nc.vector.tensor_tensor(o2, x2, c1, ALU.mult)
            nc.vector.tensor_tensor(tmp[:], x1, s1, ALU.mult)
            nc.vector.tensor_tensor(o2, o2, tmp[:], ALU.add)
            rrh = rr[:].rearrange("p (t h) -> p t h", t=2)[..., None].broadcast_to(
                [128, 2, HPC, D])
            nc.vector.tensor_tensor(dstv, dstv, rrh, ALU.mult)

            rqkb = rqkb_pool.tile([128, 2 * CW], BF16, tag="rqkb")
            nc.vector.tensor_copy(rqkb[:], rqk[:])

            for half, kp, ks in ((0, "qTp", "qTs"), (1, "kTp", "kTs")):
                o = half * CW
                ptp = tp2_pool.tile([128, 128], BF16, tag="tp2")
                nc.tensor.transpose(ptp[0:64, :], rqkb[:, o:o + D],
                                    identb[:], tile_position=(0, 0))
                nc.tensor.transpose(ptp[64:128, :], rqkb[:, o + D:o + 2 * D],
                                    identb[:], tile_position=(0, 64))
                nc.vector.tensor_copy(T[kp][:, stsl], ptp[:])
                pts = tp2_pool.tile([64, 128], BF16, tag="tp2")
                nc.tensor.transpose(pts[:], rqkb[:, o + 2 * D:o + 3 * D],
                                    identb[:])
                nc.vector.tensor_copy(T[ks][:, stsl], pts[:])

        def attn_tail(b, ps_o, slots):
            """slots: list of (head, qc) for each slot of ps_o [65, n, QC]."""
            nh = len(slots)
            o_sb = osb_pool.tile([D + 1, nh, QC], F32, tag="osb")
            nc.vector.tensor_copy(o_sb[:], ps_o[:])
            for i, (h, qc) in enumerate(slots):
                for j in range(QC // 128):
                    tpo = tp2_pool.tile([128, D + 1], F32, tag="tp2")
                    nc.tensor.transpose(
                        tpo[:], o_sb[:, i, j * 128:(j + 1) * 128],
                        identf[0:D + 1, 0:D + 1])
                    rc = rc_pool.tile([128, 1], F32, tag="rc")
                    nc.vector.reciprocal(rc[:], tpo[:, D:D + 1])
                    out_sb = out_pool.tile([128, D], F32, tag="out")
                    nc.vector.tensor_scalar_mul(out_sb[:], tpo[:, 0:D], rc[:])
                    r0 = b * S + qc * QC + j * 128
                    nc.sync.dma_start(
                        out_d[r0:r0 + 128, h * D:(h + 1) * D], out_sb[:])

        def phase2_pair_chunk(b, qc):
            """Software-pipelined: exp/PV run one k-tile behind the scores
            matmuls so the in-order PE never waits on the ACT exp result."""
            T = tiles[b]
            qcs = slice(qc * QC, (qc + 1) * QC)
            ps_o = None if (no_pv or no_exp) else po_pool.tile(
                [D + 1, 2, QC], F32, tag="po")

            def exp_of(ps_s):
                pT = pT_pool.tile([128, 2, QC], BF16, tag="pT")
                nc.scalar.activation(
                    pT[:].rearrange("p a q -> p (a q)"),
                    ps_s[:].rearrange("p a q -> p (a q)"),
                    AF.Exp, scale=SCALE)
                return pT

            def pv(pT, kt, i):
                nc.tensor.matmul(
                    ps_o[:, i, :], T["v"][:, kt, i, :], pT[:, i, :],
                    start=(kt == 0), stop=(kt == KTILES - 1))

            # PV lags the scores by TWO k-tiles: by the time the in-order PE
            # reaches pv(kt-2), its exp finished long ago, so the PE never
            # breaks its back-to-back matmul stream on an ACT semaphore.
            pend = []
            for kt in range(KTILES):
                kts = slice(kt * 128, (kt + 1) * 128)
                ps_s = ps_s_pool.tile([128, 2, QC], F32, tag="ps_s")
                nc.tensor.matmul(ps_s[:, 0, :], T["kTp"][0:64, kts],
                                 T["qTp"][0:64, qcs], start=True, stop=True)
                nc.tensor.matmul(ps_s[:, 1, :], T["kTp"][64:128, kts],
                                 T["qTp"][64:128, qcs], start=True, stop=True)
                if no_exp:
                    continue
                pend.append((exp_of(ps_s), kt))
                if len(pend) > 2 and not no_pv:
                    pT_, kt_ = pend.pop(0)
                    pv(pT_, kt_, 0)
                    pv(pT_, kt_, 1)
            if not no_pv:
                for pT_, kt_ in pend:
                    pv(pT_, kt_, 0)
                    pv(pT_, kt_, 1)
            if not (no_pv or no_exp):
                attn_tail(b, ps_o, [(0, qc), (1, qc)])

        def phase2_single_chunk(b, qc):
            T = tiles[b]
            qcs = slice(qc * QC, (qc + 1) * QC)
            ps_o = None if (no_pv or no_exp) else po_pool.tile(
                [D + 1, 1, QC], F32, tag="po")

            def exp_of(ps_s):
                pT = pT_pool.tile([128, 2, QC], BF16, tag="pT")
                nc.scalar.activation(
                    pT[:].rearrange("p a q -> p (a q)"),
                    ps_s[:].rearrange("p a q -> p (a q)"),
                    AF.Exp, scale=SCALE)
                return pT

            def pv(pT, g, i):
                kt = 2 * g + i
                nc.tensor.matmul(
                    ps_o[:, 0, :], T["v"][:, kt, 2, :], pT[:, i, :],
                    start=(kt == 0), stop=(kt == KTILES - 1))

            pend = []
            for g in range(KTILES // 2):
                ps_s = ps_s_pool.tile([128, 2, QC], F32, tag="ps_s")
                for i in range(2):
                    kt = 2 * g + i
                    nc.tensor.matmul(
                        ps_s[:, i, :], T["kTs"][:, kt * 128:(kt + 1) * 128],
                        T["qTs"][:, qcs], start=True, stop=True)
                if no_exp:
                    continue
                pend.append((exp_of(ps_s), g))
                if len(pend) > 2 and not no_pv:
                    pT_, g_ = pend.pop(0)
                    pv(pT_, g_, 0)
                    pv(pT_, g_, 1)
            if not no_pv:
                for pT_, g_ in pend:
                    pv(pT_, g_, 0)
                    pv(pT_, g_, 1)
            if not (no_pv or no_exp):
                attn_tail(b, ps_o, [(2, qc)])

        # ---- interleaved emission: phase2(b) with phase1(b+1) ----------
        do1, do2 = (1 in phases), (2 in phases)
        alloc_batch(0)
        if do1:
            for st in range(ST_PER_B):
                phase1_tile(0, st)
        for b in range(B):
            chunks = []
            if do2:
                for qc in range(NQC):
                    chunks.append(lambda b=b, qc=qc: phase2_pair_chunk(b, qc))
                    chunks.append(lambda b=b, qc=qc: phase2_single_chunk(b, qc))
            nxt = []
            if b + 1 < B:
                alloc_batch(b + 1)
                if do1:
                    nxt = [lambda st=st: phase1_tile(b + 1, st)
                           for st in range(ST_PER_B)]
            if not chunks:
                for t in nxt:
                    t()
                continue
            per = (len(nxt) + len(chunks) - 1) // len(chunks) if nxt else 0
            for i, c in enumerate(chunks):
                c()
                for t in nxt[i * per:(i + 1) * per]:
                    t()


_CACHE = {}


def _get_program():
    if "nc" not in _CACHE:
        nc = bacc.Bacc("TRN2", target_bir_lowering=False, debug=False,
                       num_devices=NCORES)
        with tile.TileContext(nc) as tc:
            _emit(tc)
        nc.compile()
        _CACHE["nc"] = nc
    return _CACHE["nc"]


def _get_runner():
    """Build (once) a cached sharded-jit callable over 8 cores."""
    if "runner" in _CACHE:
        return _CACHE["runner"]
    nc = _get_program()
    import jax
    from jax.sharding import Mesh, PartitionSpec
    try:
        from jax.experimental.shard_map import shard_map
    except ImportError:
        from jax.shard_map import shard_map
    from concourse import bass2jax
    bass2jax.install_neuronx_cc_hook()

    part_name = (nc.partition_id_tensor.name
                 if nc.partition_id_tensor is not None else None)
    in_names, out_names, out_avals, zero_shapes = [], [], [], []
    for alloc in nc.m.functions[0].allocations:
        if not isinstance(alloc, mybir.MemoryLocationSet):
            continue
        name = alloc.memorylocations[0].name
        if alloc.kind == "ExternalInput":
            if name != part_name:
                in_names.append(name)
        elif alloc.kind == "ExternalOutput":
            out_names.append(name)
            shape = tuple(alloc.tensor_shape)
            dtype = mybir.dt.np(alloc.dtype)
            out_avals.append(jax.core.ShapedArray(shape, dtype))
            zero_shapes.append((shape, dtype))
    n_params = len(in_names)
    all_names = in_names + out_names
    if part_name is not None:
        all_names = all_names + [part_name]

    def _body(*args):
        operands = list(args)
        if part_name is not None:
            operands.append(bass2jax.partition_id_tensor())
        outs = bass2jax._bass_exec_p.bind(
            *operands,
            out_avals=tuple(out_avals),
            in_names=tuple(all_names),
            out_names=tuple(out_names),
            lowering_input_output_aliases=(),
            sim_require_finite=True,
            sim_require_nnan=True,
            nc=nc,
        )
        return tuple(outs)

    devices = jax.devices()[:NCORES]
    mesh = Mesh(np.asarray(devices), ("core",))
    n_outs = len(out_names)
    donate = tuple(range(n_params, n_params + n_outs))
    in_specs = (PartitionSpec("core"),) * (n_params + n_outs)
    out_specs = (PartitionSpec("core"),) * n_outs
    sharded = jax.jit(
        shard_map(_body, mesh=mesh, in_specs=in_specs, out_specs=out_specs,
                  check_rep=False),
        donate_argnums=donate, keep_unused=True)
    _CACHE["runner"] = (sharded, in_names, out_names, zero_shapes, mesh)
    return _CACHE["runner"]


def _prep_inputs(x, cos, sin, w_qkv, b_qkv):
    """Per-core shards, concatenated along axis 0 (shard_map layout)."""
    xT = np.ascontiguousarray(x.reshape(BS, DIM).T).astype(ml_dtypes.bfloat16)
    cos32 = np.ascontiguousarray(cos, dtype=np.float32)
    sin32 = np.ascontiguousarray(sin, dtype=np.float32)
    in_maps = []
    for c in range(NCORES):
        c0 = c * CW
        w_shard = np.concatenate(
            [w_qkv[:, c0:c0 + CW], w_qkv[:, DIM + c0:DIM + c0 + CW],
             w_qkv[:, 2 * DIM + c0:2 * DIM + c0 + CW]], axis=1)
        b_shard = np.concatenate(
            [b_qkv[c0:c0 + CW], b_qkv[DIM + c0:DIM + c0 + CW],
             b_qkv[2 * DIM + c0:2 * DIM + c0 + CW]])[None, :]
        in_maps.append({
            "xT": xT,
            "w": np.ascontiguousarray(w_shard).astype(ml_dtypes.bfloat16),
            "bias": np.ascontiguousarray(b_shard).astype(ml_dtypes.bfloat16),
            "cos": cos32,
            "sin": sin32,
        })
    return in_maps


def _run(in_maps):
    sharded, in_names, out_names, zero_shapes, mesh = _get_runner()
    concat_in = [
        np.concatenate([in_maps[c][name] for c in range(NCORES)], axis=0)
        for name in in_names]
    zeros = [np.zeros((NCORES * s[0], *s[1:]), dt) for s, dt in zero_shapes]
    out_arrs = sharded(*concat_in, *zeros)
    res = {}
    for i, name in enumerate(out_names):
        arr = np.asarray(out_arrs[i])
        res[name] = arr.reshape(NCORES, arr.shape[0] // NCORES, *arr.shape[1:])
    return res


def kernel(x, cos, sin, w_qkv, b_qkv):
    in_maps = _prep_inputs(x, cos, sin, w_qkv, b_qkv)
    res = _run(in_maps)
    out = np.concatenate([res["out"][c] for c in range(NCORES)], axis=1)
    return np.ascontiguousarray(out.reshape(B, S, DIM), dtype=np.float32)

